# revision 7
# baseline (speedup 1.0000x reference)
"""MemEffEquivariantAttention TRN2 Bass kernel.

Sharding: 8 cores = 4 batches x 2 query-token halves (fully data-parallel,
no collectives). Each core computes, for its (batch, 256-token half):
scores -> +bias(masked) -> exp (no max; range-safe) -> p = e/Z * law ->
attn = p @ vf -> equivariant LN -> out_proj.

Device dataflow per core (all 16 heads):
  - scores: fp32r matmuls, lhsT=qT [96,128], rhs=kT_all[h] [96,1024]
    (expanded half of kT is host-gathered from outcell_index; v expansion
    is done ON DEVICE with dma_gather row-gather from HBM)
  - bias add: DVE (PSUM f32 + bf16 SBUF -> f32)
  - exp: ACT with accum_out => Z row-sums for free
  - u = (e * 1/Z) * law in one DVE scalar_tensor_tensor -> bf16
  - u -> uT: dma_gather SBUF-source transpose (identity idx)
  - attn: bf16 matmuls accumulating attnT [96, 256] over 8 s-chunks
  - LN: sumsq via per-head ones-matmul into [t,1] PSUM; inv = rsqrt
  - out_proj: f32r matmuls over hid chunks; per-partition scale by inv
"""
import sys
sys.path.insert(0, "/opt/trn_rl_repo")

import numpy as np
import ml_dtypes

import concourse.bacc as bacc
import concourse.tile as tile
from concourse import mybir
from concourse.bass_utils import run_bass_kernel_spmd

F32 = mybir.dt.float32
F32R = mybir.dt.float32r
BF16 = mybir.dt.bfloat16
I16 = mybir.dt.int16
AF = mybir.ActivationFunctionType
ALU = mybir.AluOpType

B, T, P, HID = 4, 512, 3, 512
HD, H = 32, 16
EXP, S = 512, 1024
TQ = 256            # query tokens per core
EPS = 1e-3
CUTOFF = 1e-5
NEG = -1e30
D = P * HD          # 96, per-head feature dim

_prog_cache = {}


def _wrap_idx(idx):
    # gpsimd wrapped layout, replicated to all 8 gpsimd cores:
    # idxs[p, s] = idx[s*16 + (p % 16)]
    n = len(idx)
    w = idx.reshape(n // 16, 16).T.astype(np.int16)
    return np.ascontiguousarray(np.tile(w, (8, 1)))


def _build_program():
    nc = bacc.Bacc("TRN2", target_bir_lowering=False, debug=False)

    qT_d = nc.dram_tensor("qT", [H, D, TQ], F32R, kind="ExternalInput").ap()
    kT_d = nc.dram_tensor("kT", [H, D, S], F32R, kind="ExternalInput").ap()
    vpk_d = nc.dram_tensor("vpk", [T, H * D], BF16, kind="ExternalInput").ap()
    bias_d = nc.dram_tensor("bias", [H, 2, 128, S], BF16, kind="ExternalInput").ap()
    law_d = nc.dram_tensor("law", [2, 128, S], BF16, kind="ExternalInput").ap()
    WT_d = nc.dram_tensor("WT", [HID, HID], F32R, kind="ExternalInput").ap()
    idv_d = nc.dram_tensor("idv", [128, 32], I16, kind="ExternalInput").ap()
    idt_d = nc.dram_tensor("idt", [128, 16], I16, kind="ExternalInput").ap()
    ones_d = nc.dram_tensor("ones96", [D, 1], BF16, kind="ExternalInput").ap()
    out_d = nc.dram_tensor("out", [TQ, P, HID], F32, kind="ExternalOutput").ap()

    with tile.TileContext(nc) as tc:
        with tc.tile_pool(name="const", bufs=1) as cp, \
             tc.tile_pool(name="work", bufs=2) as wp, \
             tc.tile_pool(name="biasp", bufs=3) as bp, \
             tc.tile_pool(name="psw", bufs=2, space="PSUM") as psw, \
             tc.tile_pool(name="psa", bufs=2, space="PSUM") as psa, \
             tc.tile_pool(name="pss", bufs=1, space="PSUM") as pss:

            # ---- constants / preload ----
            qT_t = cp.tile([D, H, TQ], F32R, tag="qT")
            kT_t = cp.tile([D, H, S], F32R, tag="kT")
            v_t = cp.tile([128, 4, H * D], BF16, tag="v")
            vg_t = cp.tile([128, 4, H * D], BF16, tag="vg")
            law_t = cp.tile([128, 2, S], BF16, tag="law")
            WT_t = cp.tile([128, 4, HID], F32R, tag="WT")
            idv_t = cp.tile([128, 32], I16, tag="idv")
            idt_t = cp.tile([128, 16], I16, tag="idt")
            ones_t = cp.tile([D, 1], BF16, tag="ones")
            X_t = cp.tile([128, P, 4, TQ], F32R, tag="X")
            eps_t = cp.tile([128, 1], F32, tag="eps")
            nc.vector.memset(eps_t[:], EPS)

            nc.sync.dma_start(out=qT_t[:], in_=qT_d.rearrange("h d t -> d h t"))
            nc.sync.dma_start(out=kT_t[:], in_=kT_d.rearrange("h d s -> d h s"))
            nc.sync.dma_start(out=v_t[:], in_=vpk_d.rearrange("(c p) d -> p c d", p=128))
            nc.sync.dma_start(out=law_t[:], in_=law_d.rearrange("r p s -> p r s"))
            nc.sync.dma_start(out=WT_t[:], in_=WT_d.rearrange("(c p) o -> p c o", p=128))
            nc.sync.dma_start(out=idv_t[:], in_=idv_d)
            nc.sync.dma_start(out=idt_t[:], in_=idt_d)
            nc.sync.dma_start(out=ones_t[:], in_=ones_d)

            # v expansion: gather token rows of vpk from HBM by outcell_index
            nc.gpsimd.dma_gather(vg_t[:], vpk_d, idv_t[:],
                                 num_idxs=EXP, num_idxs_reg=EXP,
                                 elem_size=H * D)

            ss_ps = [pss.tile([128, 1], F32, tag=f"ss{tb}", name=f"ss{tb}")
                     for tb in range(2)]

            # ---- main loop over heads ----
            for h in range(H):
                u_t = wp.tile([128, 2, S], BF16, tag="u")
                for tt in range(2):
                    bias_t = bp.tile([128, S], BF16, tag="bias")
                    nc.sync.dma_start(out=bias_t[:], in_=bias_d[h, tt])

                    w_ps = psw.tile([128, S], F32, tag="w")
                    lhs = qT_t[:, h, tt * 128:(tt + 1) * 128]
                    nc.tensor.matmul(w_ps[:, 0:512], lhs, kT_t[:, h, 0:512],
                                     start=True, stop=True)
                    nc.tensor.matmul(w_ps[:, 512:1024], lhs, kT_t[:, h, 512:1024],
                                     start=True, stop=True)

                    wsum_t = wp.tile([128, S], F32, tag="wsum")
                    nc.vector.tensor_add(wsum_t[:], w_ps[:], bias_t[:])

                    e_t = wp.tile([128, S], BF16, tag="e")
                    z_t = wp.tile([128, 1], F32, tag="z")
                    nc.scalar.activation(e_t[:], wsum_t[:], AF.Exp, accum_out=z_t[:])
                    rz_t = wp.tile([128, 1], F32, tag="rz")
                    nc.vector.reciprocal(rz_t[:], z_t[:])

                    nc.vector.scalar_tensor_tensor(
                        u_t[:, tt, :], e_t[:], rz_t[:], law_t[:, tt, :],
                        op0=ALU.mult, op1=ALU.mult)

                # transpose u -> uT chunks [s_local, t] via SBUF-source gather
                uT_t = wp.tile([128, 8, TQ], BF16, tag="uT")
                nc.gpsimd.dma_gather(uT_t[:], u_t[:], idt_t[:],
                                     num_idxs=TQ, num_idxs_reg=TQ,
                                     elem_size=S, transpose=True,
                                     sbuf_tokens_per_rank=128,
                                     sbuf_free_dim_per_rank=2 * S)

                # attn^T[h] = sum_s vf[s, d] * p[t, s]  -> [96, 256]
                at_ps = psa.tile([D, TQ], F32, tag="attn")
                for sc in range(8):
                    vsrc = v_t if sc < 4 else vg_t
                    nc.tensor.matmul(at_ps[:],
                                     vsrc[:, sc % 4, h * D:(h + 1) * D],
                                     uT_t[:, sc, :],
                                     start=(sc == 0), stop=(sc == 7))

                at_sb = wp.tile([D, TQ], F32R, tag="atsb")
                nc.scalar.activation(at_sb[:], at_ps[:], AF.Copy)

                # stash into X[(h%4)*32+j, p, h//4, t] for out_proj lhsT
                for p in range(P):
                    nc.sync.dma_start(
                        out=X_t[(h % 4) * 32:(h % 4 + 1) * 32, p, h // 4, :],
                        in_=at_sb[p * 32:(p + 1) * 32, :])

                # sumsq: sq = attnT^2 (bf16), ones-matmul accumulates [t,1]
                sq_t = wp.tile([D, TQ], BF16, tag="sq")
                nc.vector.tensor_mul(sq_t[:], at_sb[:], at_sb[:])
                for tb in range(2):
                    nc.tensor.matmul(ss_ps[tb][:],
                                     sq_t[:, tb * 128:(tb + 1) * 128],
                                     ones_t[:],
                                     start=(h == 0), stop=(h == H - 1))

            # ---- inv = 1/sqrt(mean+eps), out_proj, scale, store ----
            inv_t = []
            for tb in range(2):
                tmp_t = wp.tile([128, 1], F32, tag=f"tmp{tb}")
                nc.scalar.activation(tmp_t[:], ss_ps[tb][:], AF.Sqrt,
                                     scale=1.0 / HID, bias=eps_t[:])
                iv = wp.tile([128, 1], F32, tag=f"inv{tb}")
                nc.vector.reciprocal(iv[:], tmp_t[:])
                inv_t.append(iv)

            for p in range(P):
                for tb in range(2):
                    o_ps = psa.tile([128, HID], F32, tag="attn")
                    for ci in range(4):
                        nc.tensor.matmul(o_ps[:],
                                         X_t[:, p, ci, tb * 128:(tb + 1) * 128],
                                         WT_t[:, ci, :],
                                         start=(ci == 0), stop=(ci == 3))
                    o_sb = wp.tile([128, HID], F32, tag="osb")
                    nc.vector.tensor_scalar_mul(o_sb[:], o_ps[:], inv_t[tb][:])
                    nc.sync.dma_start(out=out_d[tb * 128:(tb + 1) * 128, p, :],
                                      in_=o_sb[:])

    nc.compile()
    return nc


def _get_program():
    if "nc" not in _prog_cache:
        _prog_cache["nc"] = _build_program()
    return _prog_cache["nc"]


def _prepare_in_maps(q, k, v, attn_bias, key_padding_mask, outcell_index,
                     local_attention_weight, expand_mask, out_proj_weight,
                     attn_ln_weight):
    q = np.asarray(q, dtype=np.float32)
    k = np.asarray(k, dtype=np.float32)
    v = np.asarray(v, dtype=np.float32)
    attn_bias = np.asarray(attn_bias, dtype=np.float32)
    kpm = np.asarray(key_padding_mask)
    idx = np.asarray(outcell_index).astype(np.int64)
    law = np.asarray(local_attention_weight, dtype=np.float32)
    emask = np.asarray(expand_mask)
    W = np.asarray(out_proj_weight, dtype=np.float32)
    lnw = np.asarray(attn_ln_weight, dtype=np.float32)

    WT = np.ascontiguousarray((W * lnw[None, :]).T)  # [hid, o], ln folded
    idt_np = _wrap_idx(np.arange(TQ, dtype=np.int16))
    ones_np = np.ones((D, 1), dtype=ml_dtypes.bfloat16)

    in_maps = []
    for c in range(8):
        b, th = c // 2, c % 2
        tsl = slice(th * TQ, (th + 1) * TQ)

        qT = np.ascontiguousarray(
            q[b, tsl].reshape(TQ, P, H, HD).transpose(2, 1, 3, 0).reshape(H, D, TQ))
        kTl = k[b].reshape(T, P, H, HD).transpose(2, 1, 3, 0).reshape(H, D, T)
        kT = np.concatenate([kTl, kTl[:, :, idx[b]]], axis=2)  # [H, D, 1024]
        vpk = v[b].reshape(T, P, H, HD).transpose(0, 2, 1, 3).reshape(T, H * D)

        bias_c = np.ascontiguousarray(attn_bias[b, :, tsl, :])  # [H, 256, S]
        kpmS = np.concatenate([kpm[b], emask[b]])               # [S]
        if kpmS.any():
            bias_c[:, :, kpmS] = NEG
        cut = law[b, tsl] <= CUTOFF                             # [256, S]
        if cut.any():
            bias_c[:, cut] = NEG

        in_maps.append(dict(
            qT=qT.astype(np.float32),
            kT=np.ascontiguousarray(kT).astype(np.float32),
            vpk=vpk.astype(ml_dtypes.bfloat16),
            bias=bias_c.reshape(H, 2, 128, S).astype(ml_dtypes.bfloat16),
            law=np.ascontiguousarray(law[b, tsl].reshape(2, 128, S)).astype(
                ml_dtypes.bfloat16),
            WT=WT,
            idv=_wrap_idx(idx[b].astype(np.int16)),
            idt=idt_np,
            ones96=ones_np,
        ))
    return in_maps


def kernel(**inputs):
    in_maps = _prepare_in_maps(**inputs)
    nc = _get_program()
    res = run_bass_kernel_spmd(nc, in_maps, list(range(8)))

    out = np.empty((B, T, P, HID), dtype=np.float32)
    for c in range(8):
        b, th = c // 2, c % 2
        out[b, th * TQ:(th + 1) * TQ] = res.results[c]["out"]
    return out
